# revision 13
# baseline (speedup 1.0000x reference)
"""GNN (MLP + 2x GCNConv + head) on 8 Trainium2 NeuronCores.

Sharding: nodes split 8 ways (12544 per core, padded 100000 -> 100352).
Per conv: transform on PE (feature-major, bf16), scale by dinv[src],
PE-transpose to node-major, AllGather of the bf16 table (4 quarter
collectives), then aggregation in G-pair groups: per (group, quarter) ONE
dma_gather call (int16 idx) pulls all edge source rows into SBUF staging
(SWDGE per-call fixed cost ~4.1us dominates, so calls are merged across
pairs); a DVE iota-compare builds the one-hot from shipped dst-position
bytes per (group, quarter); one-hot matmuls accumulate each dst tile in
PSUM across all 4 quarters; evacuation adds self-loop + bias + relu.
All edge bookkeeping (per-group chunked idx/pos streams) precomputed on
host.
"""
import os
import numpy as np

N_NODES = 100000
N_PAD = 100352          # 8 * 12544
SH = 12544              # nodes per core (98 tiles of 128)
NT = 98                 # 128-node tiles per core
NR = 4                  # src ranges = per-shard quarters (int16 idx limit)
QR = SH // NR           # 3136 rows per core-shard quarter
RW = N_PAD // NR        # 25088 rows per gathered quarter block
CHUNK = 128             # edges per matmul chunk
HID = 128
NCORES = 8
NP2 = NT // 2           # 49 tile pairs
G = 2                   # tile pairs per gather group
NPG = (NP2 + G - 1) // G  # 25 pair-groups

_cache = {}
last_results = None


def _prep(edge_index):
    src = np.asarray(edge_index[0], dtype=np.int64)
    dst = np.asarray(edge_index[1], dtype=np.int64)
    deg = np.bincount(dst, minlength=N_PAD).astype(np.float64) + 1.0
    dinv = (1.0 / np.sqrt(deg)).astype(np.float32)  # pad nodes -> 1.0

    core_of = dst // SH
    # stream groups: ((pg*NR + r)*G + pi)*2 + tp, so each (pg, r) slice of
    # the chunk stream is contiguous (one gather call per (pg, r)); within
    # it, chunks sit pair-major then tile-parity.  r is the source's quarter
    # within its core's shard (matches the quarter AllGather layout).
    NG = NPG * NR * G * 2
    cnt = np.zeros((NCORES, NG), dtype=np.int64)
    per_core = []
    for c in range(NCORES):
        m = core_of == c
        s = src[m]
        dl = dst[m] - c * SH
        t = dl // 128
        p = t // 2
        r = (s % SH) // QR
        g = ((p // G) * NR + r) * (G * 2) + (p % G) * 2 + (t % 2)
        o = np.argsort(g, kind="stable")
        s, dl, g = s[o], dl[o], g[o]
        cnt[c] = np.bincount(g, minlength=NG)
        per_core.append((s, dl, g))
    NCH = np.maximum((cnt.max(axis=0) + CHUNK - 1) // CHUNK, 1)  # [NG]
    TOTCH = int(NCH.sum())
    CHOFF = np.concatenate([[0], np.cumsum(NCH)]).astype(np.int64)  # [NG+1]
    TOT = TOTCH * CHUNK

    gidx16 = np.zeros((NCORES, 16, TOT // 16), dtype=np.int16)
    pos = np.full((NCORES, 128, TOTCH), 255.0, dtype=np.float32)
    for c in range(NCORES):
        s, dl, g = per_core[c]
        gstart = np.concatenate([[0], np.cumsum(cnt[c])])
        j_in_g = np.arange(len(g)) - gstart[g]          # position within group
        lin = CHOFF[g] * CHUNK + j_in_g                 # global stream position
        idx16 = np.zeros(TOT, dtype=np.int16)
        # row within quarter block: src core * 3136 + offset within quarter
        idx16[lin] = ((s // SH) * QR + (s % QR)).astype(np.int16)
        gidx16[c] = idx16.reshape(TOT // 16, 16).T
        pos[c, lin % 128, lin // 128] = (dl % 128).astype(np.float32)
    return dinv, NCH, CHOFF, TOTCH, gidx16, pos


def _build(NCH, CHOFF, TOTCH):
    import concourse.bacc as bacc
    import concourse.bass as bass
    import concourse.mybir as mybir
    import concourse.tile as tile
    from concourse.masks import make_identity

    f32 = mybir.dt.float32
    bf16 = mybir.dt.bfloat16
    i16 = mybir.dt.int16
    RELU = mybir.ActivationFunctionType.Relu
    COPY = mybir.ActivationFunctionType.Copy
    EQ = mybir.AluOpType.is_equal

    # chunks per pair-group and per (pg, r)
    g0_of = [pg * NR * G * 2 for pg in range(NPG)]
    nch_pg = [int(CHOFF[g0_of[pg] + NR * G * 2] - CHOFF[g0_of[pg]])
              for pg in range(NPG)]
    NCHMAX = max(nch_pg)
    NCRMAX = max(int(CHOFF[g0_of[pg] + (r + 1) * G * 2]
                     - CHOFF[g0_of[pg] + r * G * 2])
                 for pg in range(NPG) for r in range(NR))
    TOT16 = TOTCH * CHUNK // 16

    nc = bacc.Bacc("TRN2", target_bir_lowering=False, debug=False,
                   enable_asserts=False, num_devices=NCORES,
                   dynamic_dma_scratch_size=73728, num_swdge_queues=4)

    xT = nc.dram_tensor("xT", [5, SH], bf16, kind="ExternalInput")
    gidx16 = nc.dram_tensor("gidx16", [16, TOT16], i16, kind="ExternalInput")
    posd = nc.dram_tensor("posd", [128, TOTCH], bf16, kind="ExternalInput")
    iota = nc.dram_tensor("iota", [128, 128], bf16, kind="ExternalInput")
    dinv_cols = nc.dram_tensor("dinv_cols", [128, NT], f32, kind="ExternalInput")
    wts = {}
    for nm, shp, dt in [
            ("w1T", [5, 64], bf16), ("w2T", [64, 128], bf16),
            ("w3T", [128, 128], bf16), ("w4T", [128, 128], bf16),
            ("wc1T", [128, 128], bf16), ("wc2T", [128, 128], bf16),
            ("w5T", [128, 60], bf16), ("b1c", [64, 1], f32),
            ("b2c", [128, 1], f32), ("b3c", [128, 1], f32),
            ("b4c", [128, 1], f32), ("b5c", [60, 1], f32),
            ("bc1c", [128, 1], f32), ("bc2c", [128, 1], f32)]:
        wts[nm] = nc.dram_tensor(nm, shp, dt, kind="ExternalInput")
    out = nc.dram_tensor("out", [SH, 60], f32, kind="ExternalOutput")

    with tile.TileContext(nc) as tc:
        with tc.tile_pool(name="w", bufs=1) as wp, \
             tc.tile_pool(name="hs", bufs=2) as hsp, \
             tc.tile_pool(name="xs", bufs=3) as xsp, \
             tc.tile_pool(name="sm", bufs=4) as smp, \
             tc.tile_pool(name="idx", bufs=2) as idxp, \
             tc.tile_pool(name="oh", bufs=2) as ohp, \
             tc.tile_pool(name="gat", bufs=2) as gatp, \
             tc.tile_pool(name="mm", bufs=2, space="PSUM") as mmp, \
             tc.tile_pool(name="tr", bufs=1, space="PSUM") as trp, \
             tc.tile_pool(name="agg", bufs=4, space="PSUM") as aggp, \
             tc.tile_pool(name="dram", bufs=1, space="DRAM") as dramp:

            W = {}
            for nm in wts:
                W[nm] = wp.tile(list(wts[nm].shape), wts[nm].dtype, tag=nm,
                                name=nm + "_sb")
                nc.sync.dma_start(out=W[nm][:], in_=wts[nm][:])
            dinv_sb = wp.tile([128, NT], f32, tag="dinv", name="dinv_sb")
            nc.sync.dma_start(out=dinv_sb[:], in_=dinv_cols[:])
            iota_sb = wp.tile([128, 128], bf16, tag="iota", name="iota_sb")
            nc.sync.dma_start(out=iota_sb[:], in_=iota[:])
            posb_sb = wp.tile([128, TOTCH], bf16, tag="pos", name="posb_sb")
            nc.sync.dma_start(out=posb_sb[:], in_=posd[:])
            identb = wp.tile([128, 128], bf16, tag="identb", name="identb")
            make_identity(nc, identb[:])
            identf = wp.tile([128, 128], f32, tag="identf", name="identf")
            make_identity(nc, identf[:])

            ag_in = dramp.tile([SH, HID], bf16, name="ag_in")
            ag_out = [dramp.tile([RW, HID], bf16, name=f"ag_out_q{q}",
                                 addr_space="Shared") for q in range(NR)]
            ag_in2 = dramp.tile([SH, HID], bf16, name="ag_in2")
            ag_out2 = [dramp.tile([RW, HID], bf16, name=f"ag_out2_q{q}",
                                  addr_space="Shared") for q in range(NR)]
            gidx128 = dramp.tile([128, TOT16], i16, name="gidx128")

            # replicate idx stream to 8 partition stripes (8 Q7 cores)
            for k in range(8):
                nc.sync.dma_start(out=gidx128[16 * k:16 * (k + 1), :],
                                  in_=gidx16[:, :])

            slices = [(s, min(512, SH - s)) for s in range(0, SH, 512)]

            hE_d = dramp.tile([128, SH], bf16, name="hE_d")
            hF_d = dramp.tile([128, SH], bf16, name="hF_d")

            def mlp_src(s0, sw):
                # streamed MLP: x -> h1 -> h2 -> h3(+h2) -> h4(+h3), one
                # 512-column slice at a time; returns the h4 slice.
                xt = xsp.tile([5, 512], bf16, tag="xs", name="xt")
                nc.sync.dma_start(out=xt[:, :sw], in_=xT[:, s0:s0 + sw])
                ps = mmp.tile([128, 512], f32, space="PSUM", tag="mm")
                nc.tensor.matmul(ps[:64, :sw], lhsT=W["w1T"][:], rhs=xt[:5, :sw],
                                 start=True, stop=True)
                ha = hsp.tile([128, 512], bf16, tag="ha", name="ha")
                nc.scalar.activation(ha[:64, :sw], ps[:64, :sw], RELU,
                                     bias=W["b1c"][:])
                ps = mmp.tile([128, 512], f32, space="PSUM", tag="mm")
                nc.tensor.matmul(ps[:, :sw], lhsT=W["w2T"][:], rhs=ha[:64, :sw],
                                 start=True, stop=True)
                hb = hsp.tile([128, 512], bf16, tag="hb", name="hb")
                nc.scalar.activation(hb[:, :sw], ps[:, :sw], RELU,
                                     bias=W["b2c"][:])
                ps = mmp.tile([128, 512], f32, space="PSUM", tag="mm")
                nc.tensor.matmul(ps[:, :sw], lhsT=W["w3T"][:], rhs=hb[:, :sw],
                                 start=True, stop=True)
                hc = hsp.tile([128, 512], bf16, tag="hc", name="hc")
                nc.scalar.activation(hc[:, :sw], ps[:, :sw], RELU,
                                     bias=W["b3c"][:])
                nc.vector.tensor_add(hc[:, :sw], hc[:, :sw], hb[:, :sw])
                ps = mmp.tile([128, 512], f32, space="PSUM", tag="mm")
                nc.tensor.matmul(ps[:, :sw], lhsT=W["w4T"][:], rhs=hc[:, :sw],
                                 start=True, stop=True)
                hd = hsp.tile([128, 512], bf16, tag="hd", name="hd")
                nc.scalar.activation(hd[:, :sw], ps[:, :sw], RELU,
                                     bias=W["b4c"][:])
                nc.vector.tensor_add(hd[:, :sw], hd[:, :sw], hc[:, :sw])
                return hd

            qrr = 0

            def conv(src_of, wc_t, bc_c, agi, ago, h_next_d):
                nonlocal qrr
                # transform + dinv[src] scale + transpose to node-major table,
                # one 512-column slice (4 node tiles) at a time
                for s0, sw in slices:
                    hsl = src_of(s0, sw)
                    ps = mmp.tile([128, 512], f32, space="PSUM", tag="mm")
                    nc.tensor.matmul(ps[:, :sw], lhsT=wc_t[:], rhs=hsl[:, :sw],
                                     start=True, stop=True)
                    gs = hsp.tile([128, 512], bf16, tag="gs", name="gs")
                    nc.scalar.activation(gs[:, :sw], ps[:, :sw], COPY)
                    for k in range(sw // 128):
                        t = s0 // 128 + k
                        pt = trp.tile([128, 128], bf16, space="PSUM",
                                      tag="trb")
                        nc.tensor.transpose(out=pt[:],
                                            in_=gs[:, k * 128:(k + 1) * 128],
                                            identity=identb[:])
                        gn = smp.tile([128, 128], bf16, tag="gn", name="gn")
                        nc.scalar.activation(gn[:], pt[:], COPY,
                                             scale=dinv_sb[:, t:t + 1])
                        nc.sync.dma_start(out=agi[t * 128:(t + 1) * 128, :],
                                          in_=gn[:])
                # 4 quarter AllGathers: block q holds every core's q-th
                # shard quarter (concat by core), so gathers for quarter q
                # only wait on collective q.
                for q in range(NR):
                    nc.gpsimd.collective_compute(
                        "AllGather", mybir.AluOpType.bypass,
                        replica_groups=[list(range(NCORES))],
                        ins=[agi[q * QR:(q + 1) * QR, :].opt()],
                        outs=[ago[q][:, :].opt()],
                    )
                # aggregation per pair-group: one gather call per (pg, r);
                # PSUM accumulators for the group's 2G tiles live across all
                # 4 quarters.
                for pg in range(NPG):
                    g0 = g0_of[pg]
                    c0 = int(CHOFF[g0])
                    nch = nch_pg[pg]
                    npairs = min(G, NP2 - pg * G)
                    tiles = [pg * G * 2 + k for k in range(npairs * 2)]
                    ist = idxp.tile([128, NCHMAX * 8], i16, tag="idx",
                                    name="ist")
                    nc.sync.dma_start(out=ist[:, :nch * 8],
                                      in_=gidx128[:, c0 * 8:(c0 + nch) * 8])
                    gst = gatp.tile([128, NCHMAX, 128], bf16, tag="g",
                                    name="gst")
                    for r in range(NR):
                        cr0 = int(CHOFF[g0 + r * G * 2]) - c0
                        ncr = int(CHOFF[g0 + (r + 1) * G * 2]
                                  - CHOFF[g0 + r * G * 2])
                        nc.gpsimd.dma_gather(
                            gst[:, cr0:cr0 + ncr, :],
                            ago[r][:, :],
                            ist[:, cr0 * 8:(cr0 + ncr) * 8],
                            ncr * CHUNK, ncr * CHUNK, HID,
                            queue_num=qrr % 4, single_packet=False)
                        qrr += 1
                    # open PSUM accumulators with the self-loop matmul
                    pa = {}
                    for t in tiles:
                        gl = smp.tile([128, 128], bf16, tag="gl", name="gl")
                        nc.sync.dma_start(out=gl[:],
                                          in_=agi[t * 128:(t + 1) * 128, :])
                        pa[t] = aggp.tile([128, 128], f32, space="PSUM",
                                          tag="agg", name=f"pa{t % 8}")
                        nc.tensor.matmul(pa[t][:], lhsT=identb[:], rhs=gl[:],
                                         start=True, stop=False)
                    # last (r, chunk) per tile to close the accumulation
                    last_of = {}
                    for t in tiles:
                        k = t - pg * G * 2
                        for r in range(NR):
                            g = g0 + r * G * 2 + k
                            if CHOFF[g + 1] > CHOFF[g]:
                                last_of[t] = (r, int(CHOFF[g + 1]) - 1)
                    for r in range(NR):
                        cr0 = int(CHOFF[g0 + r * G * 2]) - c0
                        ncr = int(CHOFF[g0 + (r + 1) * G * 2]
                                  - CHOFF[g0 + r * G * 2])
                        # one-hot for this (pg, r) chunk range
                        oh = ohp.tile([128, NCRMAX * 128], bf16, tag="oh",
                                      name="oh")
                        oh3 = oh[:, :ncr * 128].rearrange("p (c f) -> p c f",
                                                          c=ncr)
                        nc.vector.tensor_tensor(
                            out=oh3,
                            in0=posb_sb[:, c0 + cr0:c0 + cr0 + ncr]
                                .unsqueeze(2).to_broadcast([128, ncr, 128]),
                            in1=iota_sb[:].unsqueeze(1)
                                .to_broadcast([128, ncr, 128]),
                            op=EQ)
                        for t in tiles:
                            k = t - pg * G * 2
                            g = g0 + r * G * 2 + k
                            for ci in range(int(CHOFF[g]), int(CHOFF[g + 1])):
                                lco = ci - c0
                                nc.tensor.matmul(
                                    pa[t][:],
                                    lhsT=oh[:, (lco - cr0) * 128:
                                            (lco - cr0 + 1) * 128],
                                    rhs=gst[:, lco, :].squeeze(),
                                    start=False,
                                    stop=(last_of[t] == (r, ci)))
                    # evacuate: relu(agg * dinv[dst] + bias), feat-major
                    for t in tiles:
                        ev = smp.tile([128, 128], bf16, tag="ev", name="ev")
                        nc.scalar.activation(ev[:], pa[t][:], COPY,
                                             scale=dinv_sb[:, t:t + 1])
                        ptE = trp.tile([128, 128], bf16, space="PSUM",
                                       tag="trb")
                        nc.tensor.transpose(out=ptE[:], in_=ev[:],
                                            identity=identb[:])
                        hn = smp.tile([128, 128], bf16, tag="hn", name="hn")
                        nc.scalar.activation(hn[:], ptE[:], RELU, bias=bc_c[:])
                        nc.sync.dma_start(
                            out=h_next_d[:, t * 128:(t + 1) * 128], in_=hn[:])

            conv(mlp_src, W["wc1T"], W["bc1c"], ag_in, ag_out, hE_d)

            def he_src(s0, sw):
                hsl = hsp.tile([128, 512], bf16, tag="he", name="he")
                nc.sync.dma_start(out=hsl[:, :sw], in_=hE_d[:, s0:s0 + sw])
                return hsl

            conv(he_src, W["wc2T"], W["bc2c"], ag_in2, ag_out2, hF_d)

            # final head: out = h6 @ W5.T + b5  -> [SH, 60]
            for s0, sw in slices:
                hsl = hsp.tile([128, 512], bf16, tag="he", name="hfs")
                nc.sync.dma_start(out=hsl[:, :sw], in_=hF_d[:, s0:s0 + sw])
                ps = mmp.tile([128, 512], f32, space="PSUM", tag="mm")
                nc.tensor.matmul(ps[:60, :sw], lhsT=W["w5T"][:],
                                 rhs=hsl[:, :sw], start=True, stop=True)
                of = xsp.tile([60, 512], f32, tag="of", name="of")
                nc.vector.tensor_scalar_add(of[:, :sw], ps[:60, :sw],
                                            W["b5c"][:])
                for q in range(0, sw, 128):
                    qw = min(128, sw - q)
                    pt = trp.tile([128, 128], f32, space="PSUM", tag="tr")
                    nc.tensor.transpose(out=pt[:qw, :60], in_=of[:60, q:q + qw],
                                        identity=identf[:60, :60])
                    on = smp.tile([128, 60], f32, tag="on", name="on")
                    nc.vector.tensor_copy(on[:qw, :], pt[:qw, :60])
                    nc.sync.dma_start(out=out[s0 + q:s0 + q + qw, :],
                                      in_=on[:qw, :])
    nc.compile()
    return nc


def kernel(x, edge_index, W1, b1, W2, b2, W3, b3, W4, b4,
           Wc1, bc1, Wc2, bc2, W5, b5):
    import ml_dtypes
    from concourse.bass_utils import run_bass_kernel_spmd

    bf = ml_dtypes.bfloat16
    x = np.asarray(x, dtype=np.float32)
    key = "k"
    if key not in _cache:
        dinv, NCH, CHOFF, TOTCH, gidx16, posf = _prep(np.asarray(edge_index))
        nc = _build(NCH, CHOFF, TOTCH)
        _cache[key] = (dinv, gidx16, posf, nc)
    dinv, gidx16, posf, nc = _cache[key]

    xp = np.zeros((N_PAD, 5), dtype=np.float32)
    xp[:N_NODES] = x
    iota = np.tile(np.arange(128, dtype=np.float32)[None, :],
                   (128, 1)).astype(bf)
    in_maps = []
    for c in range(NCORES):
        sl = slice(c * SH, (c + 1) * SH)
        m = {
            "xT": np.ascontiguousarray(xp[sl].T).astype(bf),
            "gidx16": gidx16[c],
            "posd": posf[c].astype(__import__("ml_dtypes").bfloat16),
            "iota": iota,
            "dinv_cols": np.ascontiguousarray(
                dinv[sl].reshape(NT, 128).T),
            "w1T": np.ascontiguousarray(np.asarray(W1, np.float32).T).astype(bf),
            "w2T": np.ascontiguousarray(np.asarray(W2, np.float32).T).astype(bf),
            "w3T": np.ascontiguousarray(np.asarray(W3, np.float32).T).astype(bf),
            "w4T": np.ascontiguousarray(np.asarray(W4, np.float32).T).astype(bf),
            "wc1T": np.ascontiguousarray(np.asarray(Wc1, np.float32).T).astype(bf),
            "wc2T": np.ascontiguousarray(np.asarray(Wc2, np.float32).T).astype(bf),
            "w5T": np.ascontiguousarray(np.asarray(W5, np.float32).T).astype(bf),
            "b1c": np.asarray(b1, np.float32)[:, None],
            "b2c": np.asarray(b2, np.float32)[:, None],
            "b3c": np.asarray(b3, np.float32)[:, None],
            "b4c": np.asarray(b4, np.float32)[:, None],
            "b5c": np.asarray(b5, np.float32)[:, None],
            "bc1c": np.asarray(bc1, np.float32)[:, None],
            "bc2c": np.asarray(bc2, np.float32)[:, None],
        }
        in_maps.append(m)
    global last_results
    res = run_bass_kernel_spmd(nc, in_maps, list(range(NCORES)),
                               trace=bool(os.environ.get("KERNEL_TRACE")))
    last_results = res
    outs = [res.results[c]["out"] for c in range(NCORES)]
    return np.concatenate(outs, axis=0)[:N_NODES]


# revision 14
# speedup vs baseline: 1.0397x; 1.0397x over previous
"""GNN (MLP + 2x GCNConv + head) on 8 Trainium2 NeuronCores.

Sharding: nodes split 8 ways (12544 per core, padded 100000 -> 100352).
Per conv: transform on PE (feature-major, bf16), scale by dinv[src],
PE-transpose to node-major, AllGather of the bf16 table (4 quarter
collectives), then aggregation in G-pair groups: per (group, quarter) ONE
dma_gather call (int16 idx) pulls all edge source rows into SBUF staging
(SWDGE per-call fixed cost ~4.1us dominates, so calls are merged across
pairs); the one-hot scatter matrices are PREBUILT ON HOST in fp8 (edge
structure is identical for both convs) and DMA-loaded per (group,
quarter), freeing the vector engine entirely; one-hot matmuls accumulate
each dst tile in PSUM across all 4 quarters; evacuation adds self-loop +
bias + relu.
"""
import os
import numpy as np

N_NODES = 100000
N_PAD = 100352          # 8 * 12544
SH = 12544              # nodes per core (98 tiles of 128)
NT = 98                 # 128-node tiles per core
NR = 4                  # src ranges = per-shard quarters (int16 idx limit)
QR = SH // NR           # 3136 rows per core-shard quarter
RW = N_PAD // NR        # 25088 rows per gathered quarter block
CHUNK = 128             # edges per matmul chunk
HID = 128
NCORES = 8
NP2 = NT // 2           # 49 tile pairs
G = 2                   # tile pairs per gather group
NPG = (NP2 + G - 1) // G  # 25 pair-groups

_cache = {}
last_results = None


def _prep(edge_index):
    import ml_dtypes
    src = np.asarray(edge_index[0], dtype=np.int64)
    dst = np.asarray(edge_index[1], dtype=np.int64)
    deg = np.bincount(dst, minlength=N_PAD).astype(np.float64) + 1.0
    dinv = (1.0 / np.sqrt(deg)).astype(np.float32)  # pad nodes -> 1.0

    core_of = dst // SH
    # stream groups: ((pg*NR + r)*G + pi)*2 + tp, so each (pg, r) slice of
    # the chunk stream is contiguous (one gather call per (pg, r)); within
    # it, chunks sit pair-major then tile-parity.  r is the source's quarter
    # within its core's shard (matches the quarter AllGather layout).
    NG = NPG * NR * G * 2
    cnt = np.zeros((NCORES, NG), dtype=np.int64)
    per_core = []
    for c in range(NCORES):
        m = core_of == c
        s = src[m]
        dl = dst[m] - c * SH
        t = dl // 128
        p = t // 2
        r = (s % SH) // QR
        g = ((p // G) * NR + r) * (G * 2) + (p % G) * 2 + (t % 2)
        o = np.argsort(g, kind="stable")
        s, dl, g = s[o], dl[o], g[o]
        cnt[c] = np.bincount(g, minlength=NG)
        per_core.append((s, dl, g))
    NCH = np.maximum((cnt.max(axis=0) + CHUNK - 1) // CHUNK, 1)  # [NG]
    TOTCH = int(NCH.sum())
    CHOFF = np.concatenate([[0], np.cumsum(NCH)]).astype(np.int64)  # [NG+1]
    TOT = TOTCH * CHUNK

    gidx16 = np.zeros((NCORES, 16, TOT // 16), dtype=np.int16)
    ohb = np.zeros((NCORES, 128, TOTCH * 128), dtype=ml_dtypes.float8_e4m3)
    one8 = ml_dtypes.float8_e4m3(1.0)
    for c in range(NCORES):
        s, dl, g = per_core[c]
        gstart = np.concatenate([[0], np.cumsum(cnt[c])])
        j_in_g = np.arange(len(g)) - gstart[g]          # position within group
        lin = CHOFF[g] * CHUNK + j_in_g                 # global stream position
        idx16 = np.zeros(TOT, dtype=np.int16)
        # row within quarter block: src core * 3136 + offset within quarter
        idx16[lin] = ((s // SH) * QR + (s % QR)).astype(np.int16)
        gidx16[c] = idx16.reshape(TOT // 16, 16).T
        # one-hot scatter matrix: partition = edge slot in chunk, column =
        # chunk * 128 + dst slot within tile
        ohb[c][lin % 128, (lin // 128) * 128 + (dl % 128)] = one8
    return dinv, NCH, CHOFF, TOTCH, gidx16, ohb


def _build(NCH, CHOFF, TOTCH):
    import concourse.bacc as bacc
    import concourse.bass as bass
    import concourse.mybir as mybir
    import concourse.tile as tile
    from concourse.masks import make_identity

    f32 = mybir.dt.float32
    bf16 = mybir.dt.bfloat16
    fp8 = mybir.dt.float8e4
    i16 = mybir.dt.int16
    RELU = mybir.ActivationFunctionType.Relu
    COPY = mybir.ActivationFunctionType.Copy

    # chunks per pair-group and per (pg, r)
    g0_of = [pg * NR * G * 2 for pg in range(NPG)]
    nch_pg = [int(CHOFF[g0_of[pg] + NR * G * 2] - CHOFF[g0_of[pg]])
              for pg in range(NPG)]
    NCHMAX = max(nch_pg)
    NCRMAX = max(int(CHOFF[g0_of[pg] + (r + 1) * G * 2]
                     - CHOFF[g0_of[pg] + r * G * 2])
                 for pg in range(NPG) for r in range(NR))
    TOT16 = TOTCH * CHUNK // 16

    nc = bacc.Bacc("TRN2", target_bir_lowering=False, debug=False,
                   enable_asserts=False, num_devices=NCORES,
                   dynamic_dma_scratch_size=32768, num_swdge_queues=4)

    xT = nc.dram_tensor("xT", [5, SH], bf16, kind="ExternalInput")
    gidx16 = nc.dram_tensor("gidx16", [16, TOT16], i16, kind="ExternalInput")
    ohd = nc.dram_tensor("ohd", [128, TOTCH * 128], fp8, kind="ExternalInput")
    dinv_cols = nc.dram_tensor("dinv_cols", [128, NT], f32, kind="ExternalInput")
    wts = {}
    for nm, shp, dt in [
            ("w1T", [5, 64], bf16), ("w2T", [64, 128], bf16),
            ("w3T", [128, 128], bf16), ("w4T", [128, 128], bf16),
            ("wc1T", [128, 128], bf16), ("wc2T", [128, 128], bf16),
            ("w5T", [128, 60], bf16), ("b1c", [64, 1], f32),
            ("b2c", [128, 1], f32), ("b3c", [128, 1], f32),
            ("b4c", [128, 1], f32), ("b5c", [60, 1], f32),
            ("bc1c", [128, 1], f32), ("bc2c", [128, 1], f32)]:
        wts[nm] = nc.dram_tensor(nm, shp, dt, kind="ExternalInput")
    out = nc.dram_tensor("out", [SH, 60], f32, kind="ExternalOutput")

    with tile.TileContext(nc) as tc:
        with tc.tile_pool(name="w", bufs=1) as wp, \
             tc.tile_pool(name="act", bufs=2) as actp, \
             tc.tile_pool(name="xs", bufs=3) as xsp, \
             tc.tile_pool(name="sm", bufs=4) as smp, \
             tc.tile_pool(name="idx", bufs=3) as idxp, \
             tc.tile_pool(name="oh", bufs=4) as ohp, \
             tc.tile_pool(name="gat", bufs=2) as gatp, \
             tc.tile_pool(name="mm", bufs=2, space="PSUM") as mmp, \
             tc.tile_pool(name="tr", bufs=1, space="PSUM") as trp, \
             tc.tile_pool(name="agg", bufs=4, space="PSUM") as aggp, \
             tc.tile_pool(name="dram", bufs=1, space="DRAM") as dramp:

            W = {}
            for nm in wts:
                W[nm] = wp.tile(list(wts[nm].shape), wts[nm].dtype, tag=nm,
                                name=nm + "_sb")
                nc.sync.dma_start(out=W[nm][:], in_=wts[nm][:])
            dinv_sb = wp.tile([128, NT], f32, tag="dinv", name="dinv_sb")
            nc.sync.dma_start(out=dinv_sb[:], in_=dinv_cols[:])
            identb = wp.tile([128, 128], bf16, tag="identb", name="identb")
            make_identity(nc, identb[:])
            identf = wp.tile([128, 128], f32, tag="identf", name="identf")
            make_identity(nc, identf[:])

            ag_in = dramp.tile([SH, HID], bf16, name="ag_in")
            ag_out = [dramp.tile([RW, HID], bf16, name=f"ag_out_q{q}",
                                 addr_space="Shared") for q in range(NR)]
            ag_in2 = dramp.tile([SH, HID], bf16, name="ag_in2")
            ag_out2 = [dramp.tile([RW, HID], bf16, name=f"ag_out2_q{q}",
                                  addr_space="Shared") for q in range(NR)]
            gidx128 = dramp.tile([128, TOT16], i16, name="gidx128")

            # replicate idx stream to 8 partition stripes (8 Q7 cores)
            for k in range(8):
                nc.sync.dma_start(out=gidx128[16 * k:16 * (k + 1), :],
                                  in_=gidx16[:, :])

            slices = [(s, min(512, SH - s)) for s in range(0, SH, 512)]

            def mlp_layer(dst_t, w_t, b_t, src_t, kin, kout, resid=None):
                for s0, sw in slices:
                    ps = mmp.tile([128, 512], f32, space="PSUM", tag="mm")
                    nc.tensor.matmul(ps[:kout, :sw], lhsT=w_t[:],
                                     rhs=src_t[:kin, s0:s0 + sw],
                                     start=True, stop=True)
                    nc.scalar.activation(dst_t[:kout, s0:s0 + sw],
                                         ps[:kout, :sw], RELU, bias=b_t[:])
                    if resid is not None:
                        nc.vector.tensor_add(dst_t[:kout, s0:s0 + sw],
                                             dst_t[:kout, s0:s0 + sw],
                                             resid[:kout, s0:s0 + sw])

            # ---- MLP (feature-major, bf16) ----
            hA = actp.tile([128, SH], bf16, tag="act", name="hA")
            for s0, sw in slices:
                xt = xsp.tile([5, 512], bf16, tag="xs", name="xt")
                nc.sync.dma_start(out=xt[:, :sw], in_=xT[:, s0:s0 + sw])
                ps = mmp.tile([128, 512], f32, space="PSUM", tag="mm")
                nc.tensor.matmul(ps[:64, :sw], lhsT=W["w1T"][:], rhs=xt[:5, :sw],
                                 start=True, stop=True)
                nc.scalar.activation(hA[:64, s0:s0 + sw], ps[:64, :sw], RELU,
                                     bias=W["b1c"][:])
            hB = actp.tile([128, SH], bf16, tag="act", name="hB")
            mlp_layer(hB, W["w2T"], W["b2c"], hA, 64, 128)             # h2
            hC = actp.tile([128, SH], bf16, tag="act", name="hC")
            mlp_layer(hC, W["w3T"], W["b3c"], hB, 128, 128, resid=hB)  # h3
            hD = actp.tile([128, SH], bf16, tag="act", name="hD")
            mlp_layer(hD, W["w4T"], W["b4c"], hC, 128, 128, resid=hC)  # h4

            qrr = 0

            def conv(h_fm, wc_t, bc_c, agi, ago, h_next):
                nonlocal qrr
                # transform + dinv[src] scale + transpose to node-major table
                g_fm = actp.tile([128, SH], bf16, tag="act", name="g_fm")
                for s0, sw in slices:
                    ps = mmp.tile([128, 512], f32, space="PSUM", tag="mm")
                    nc.tensor.matmul(ps[:, :sw], lhsT=wc_t[:],
                                     rhs=h_fm[:, s0:s0 + sw], start=True,
                                     stop=True)
                    nc.scalar.activation(g_fm[:, s0:s0 + sw], ps[:, :sw], COPY)
                for t in range(NT):
                    pt = trp.tile([128, 128], bf16, space="PSUM", tag="trb")
                    nc.tensor.transpose(out=pt[:],
                                        in_=g_fm[:, t * 128:(t + 1) * 128],
                                        identity=identb[:])
                    gn = smp.tile([128, 128], bf16, tag="gn", name="gn")
                    nc.scalar.activation(gn[:], pt[:], COPY,
                                         scale=dinv_sb[:, t:t + 1])
                    nc.sync.dma_start(out=agi[t * 128:(t + 1) * 128, :],
                                      in_=gn[:])
                # 4 quarter AllGathers: block q holds every core's q-th
                # shard quarter (concat by core), so gathers for quarter q
                # only wait on collective q.
                for q in range(NR):
                    nc.gpsimd.collective_compute(
                        "AllGather", mybir.AluOpType.bypass,
                        replica_groups=[list(range(NCORES))],
                        ins=[agi[q * QR:(q + 1) * QR, :].opt()],
                        outs=[ago[q][:, :].opt()],
                    )
                # aggregation per pair-group: one gather call per (pg, r);
                # PSUM accumulators for the group's 2G tiles live across all
                # 4 quarters.
                for pg in range(NPG):
                    g0 = g0_of[pg]
                    c0 = int(CHOFF[g0])
                    nch = nch_pg[pg]
                    npairs = min(G, NP2 - pg * G)
                    tiles = [pg * G * 2 + k for k in range(npairs * 2)]
                    ist = idxp.tile([128, NCHMAX * 8], i16, tag="idx",
                                    name="ist")
                    nc.sync.dma_start(out=ist[:, :nch * 8],
                                      in_=gidx128[:, c0 * 8:(c0 + nch) * 8])
                    gst = gatp.tile([128, NCHMAX, 128], bf16, tag="g",
                                    name="gst")
                    for r in range(NR):
                        cr0 = int(CHOFF[g0 + r * G * 2]) - c0
                        ncr = int(CHOFF[g0 + (r + 1) * G * 2]
                                  - CHOFF[g0 + r * G * 2])
                        nc.gpsimd.dma_gather(
                            gst[:, cr0:cr0 + ncr, :],
                            ago[r][:, :],
                            ist[:, cr0 * 8:(cr0 + ncr) * 8],
                            ncr * CHUNK, ncr * CHUNK, HID,
                            queue_num=qrr % 4, single_packet=False)
                        qrr += 1
                    # open PSUM accumulators with the self-loop matmul
                    pa = {}
                    for t in tiles:
                        gl = smp.tile([128, 128], bf16, tag="gl", name="gl")
                        nc.sync.dma_start(out=gl[:],
                                          in_=agi[t * 128:(t + 1) * 128, :])
                        pa[t] = aggp.tile([128, 128], f32, space="PSUM",
                                          tag="agg", name=f"pa{t % 8}")
                        nc.tensor.matmul(pa[t][:], lhsT=identb[:], rhs=gl[:],
                                         start=True, stop=False)
                    # last (r, chunk) per tile to close the accumulation
                    last_of = {}
                    for t in tiles:
                        k = t - pg * G * 2
                        for r in range(NR):
                            g = g0 + r * G * 2 + k
                            if CHOFF[g + 1] > CHOFF[g]:
                                last_of[t] = (r, int(CHOFF[g + 1]) - 1)
                    for r in range(NR):
                        cr0 = int(CHOFF[g0 + r * G * 2]) - c0
                        ncr = int(CHOFF[g0 + (r + 1) * G * 2]
                                  - CHOFF[g0 + r * G * 2])
                        # host-prebuilt fp8 one-hot for this (pg, r) range
                        oh = ohp.tile([128, NCRMAX * 128], fp8, tag="oh",
                                      name="oh")
                        nc.sync.dma_start(
                            out=oh[:, :ncr * 128],
                            in_=ohd[:, (c0 + cr0) * 128:
                                    (c0 + cr0 + ncr) * 128])
                        for t in tiles:
                            k = t - pg * G * 2
                            g = g0 + r * G * 2 + k
                            for ci in range(int(CHOFF[g]), int(CHOFF[g + 1])):
                                lco = ci - c0
                                nc.tensor.matmul(
                                    pa[t][:],
                                    lhsT=oh[:, (lco - cr0) * 128:
                                            (lco - cr0 + 1) * 128],
                                    rhs=gst[:, lco, :].squeeze(),
                                    start=False,
                                    stop=(last_of[t] == (r, ci)))
                    # evacuate: relu(agg * dinv[dst] + bias), feat-major
                    for t in tiles:
                        ev = smp.tile([128, 128], bf16, tag="ev", name="ev")
                        nc.scalar.activation(ev[:], pa[t][:], COPY,
                                             scale=dinv_sb[:, t:t + 1])
                        ptE = trp.tile([128, 128], bf16, space="PSUM",
                                       tag="trb")
                        nc.tensor.transpose(out=ptE[:], in_=ev[:],
                                            identity=identb[:])
                        nc.scalar.activation(h_next[:, t * 128:(t + 1) * 128],
                                             ptE[:], RELU, bias=bc_c[:])

            hE = actp.tile([128, SH], bf16, tag="act", name="hE")
            conv(hD, W["wc1T"], W["bc1c"], ag_in, ag_out, hE)

            hF = actp.tile([128, SH], bf16, tag="act", name="hF")
            conv(hE, W["wc2T"], W["bc2c"], ag_in2, ag_out2, hF)

            # final head: out = h6 @ W5.T + b5  -> [SH, 60]
            for s0, sw in slices:
                ps = mmp.tile([128, 512], f32, space="PSUM", tag="mm")
                nc.tensor.matmul(ps[:60, :sw], lhsT=W["w5T"][:],
                                 rhs=hF[:, s0:s0 + sw], start=True, stop=True)
                of = xsp.tile([60, 512], f32, tag="of", name="of")
                nc.vector.tensor_scalar_add(of[:, :sw], ps[:60, :sw],
                                            W["b5c"][:])
                for q in range(0, sw, 128):
                    qw = min(128, sw - q)
                    pt = trp.tile([128, 128], f32, space="PSUM", tag="tr")
                    nc.tensor.transpose(out=pt[:qw, :60], in_=of[:60, q:q + qw],
                                        identity=identf[:60, :60])
                    on = smp.tile([128, 60], f32, tag="on", name="on")
                    nc.vector.tensor_copy(on[:qw, :], pt[:qw, :60])
                    nc.sync.dma_start(out=out[s0 + q:s0 + q + qw, :],
                                      in_=on[:qw, :])
    nc.compile()
    return nc


def kernel(x, edge_index, W1, b1, W2, b2, W3, b3, W4, b4,
           Wc1, bc1, Wc2, bc2, W5, b5):
    import ml_dtypes
    from concourse.bass_utils import run_bass_kernel_spmd

    bf = ml_dtypes.bfloat16
    x = np.asarray(x, dtype=np.float32)
    key = "k"
    if key not in _cache:
        dinv, NCH, CHOFF, TOTCH, gidx16, ohb = _prep(np.asarray(edge_index))
        nc = _build(NCH, CHOFF, TOTCH)
        _cache[key] = (dinv, gidx16, ohb, nc)
    dinv, gidx16, ohb, nc = _cache[key]

    xp = np.zeros((N_PAD, 5), dtype=np.float32)
    xp[:N_NODES] = x
    in_maps = []
    for c in range(NCORES):
        sl = slice(c * SH, (c + 1) * SH)
        m = {
            "xT": np.ascontiguousarray(xp[sl].T).astype(bf),
            "gidx16": gidx16[c],
            "ohd": ohb[c],
            "dinv_cols": np.ascontiguousarray(
                dinv[sl].reshape(NT, 128).T),
            "w1T": np.ascontiguousarray(np.asarray(W1, np.float32).T).astype(bf),
            "w2T": np.ascontiguousarray(np.asarray(W2, np.float32).T).astype(bf),
            "w3T": np.ascontiguousarray(np.asarray(W3, np.float32).T).astype(bf),
            "w4T": np.ascontiguousarray(np.asarray(W4, np.float32).T).astype(bf),
            "wc1T": np.ascontiguousarray(np.asarray(Wc1, np.float32).T).astype(bf),
            "wc2T": np.ascontiguousarray(np.asarray(Wc2, np.float32).T).astype(bf),
            "w5T": np.ascontiguousarray(np.asarray(W5, np.float32).T).astype(bf),
            "b1c": np.asarray(b1, np.float32)[:, None],
            "b2c": np.asarray(b2, np.float32)[:, None],
            "b3c": np.asarray(b3, np.float32)[:, None],
            "b4c": np.asarray(b4, np.float32)[:, None],
            "b5c": np.asarray(b5, np.float32)[:, None],
            "bc1c": np.asarray(bc1, np.float32)[:, None],
            "bc2c": np.asarray(bc2, np.float32)[:, None],
        }
        in_maps.append(m)
    global last_results
    res = run_bass_kernel_spmd(nc, in_maps, list(range(NCORES)),
                               trace=bool(os.environ.get("KERNEL_TRACE")))
    last_results = res
    outs = [res.results[c]["out"] for c in range(NCORES)]
    return np.concatenate(outs, axis=0)[:N_NODES]


# revision 15
# speedup vs baseline: 1.1612x; 1.1169x over previous
"""GNN (MLP + 2x GCNConv + head) on 8 Trainium2 NeuronCores.

Sharding: nodes split 8 ways (12544 per core, padded 100000 -> 100352).
Per conv: transform on PE (feature-major, bf16), scale by dinv[src],
PE-transpose to node-major, ONE full AllGather of the bf16 table (the
quarter-pipelined variant contends with the SWDGE gathers on the DMA
engines), then aggregation in G-pair groups: per (group, range) ONE
dma_gather call (int16 idx over a 25088-row window of the core-concat
table); a DVE iota-compare builds the one-hot from shipped dst-position
bytes; one-hot matmuls accumulate each dst tile in PSUM across all 4
ranges; evacuation adds self-loop + bias + relu.  The next stage's
transform (conv2 / head) is emitted interleaved into the aggregation
loop (pair-group pg produces exactly the 4 node tiles of 512-col slice
pg), hiding it under the SWDGE-bound aggregation phase.
"""
import os
import numpy as np

N_NODES = 100000
N_PAD = 100352          # 8 * 12544
SH = 12544              # nodes per core (98 tiles of 128)
NT = 98                 # 128-node tiles per core
NR = 4                  # src ranges (int16 idx limit)
RW = N_PAD // NR        # 25088 rows per range window
CHUNK = 128             # edges per matmul chunk
HID = 128
NCORES = 8
NP2 = NT // 2           # 49 tile pairs
G = 2                   # tile pairs per gather group
NPG = (NP2 + G - 1) // G  # 25 pair-groups

_cache = {}
last_results = None


def _prep(edge_index):
    src = np.asarray(edge_index[0], dtype=np.int64)
    dst = np.asarray(edge_index[1], dtype=np.int64)
    deg = np.bincount(dst, minlength=N_PAD).astype(np.float64) + 1.0
    dinv = (1.0 / np.sqrt(deg)).astype(np.float32)  # pad nodes -> 1.0

    core_of = dst // SH
    # stream groups: ((pg*NR + r)*G + pi)*2 + tp, so each (pg, r) slice of
    # the chunk stream is contiguous (one gather call per (pg, r)); within
    # it, chunks sit pair-major then tile-parity.  r is the source row's
    # 25088-row window in the core-concat AllGather table (row = src id).
    NG = NPG * NR * G * 2
    cnt = np.zeros((NCORES, NG), dtype=np.int64)
    per_core = []
    for c in range(NCORES):
        m = core_of == c
        s = src[m]
        dl = dst[m] - c * SH
        t = dl // 128
        p = t // 2
        r = s // RW
        g = ((p // G) * NR + r) * (G * 2) + (p % G) * 2 + (t % 2)
        o = np.argsort(g, kind="stable")
        s, dl, g = s[o], dl[o], g[o]
        cnt[c] = np.bincount(g, minlength=NG)
        per_core.append((s, dl, g))
    NCH = np.maximum((cnt.max(axis=0) + CHUNK - 1) // CHUNK, 1)  # [NG]
    TOTCH = int(NCH.sum())
    CHOFF = np.concatenate([[0], np.cumsum(NCH)]).astype(np.int64)  # [NG+1]
    TOT = TOTCH * CHUNK

    gidx16 = np.zeros((NCORES, 16, TOT // 16), dtype=np.int16)
    pos = np.full((NCORES, 128, TOTCH), 255.0, dtype=np.float32)
    for c in range(NCORES):
        s, dl, g = per_core[c]
        gstart = np.concatenate([[0], np.cumsum(cnt[c])])
        j_in_g = np.arange(len(g)) - gstart[g]          # position within group
        lin = CHOFF[g] * CHUNK + j_in_g                 # global stream position
        idx16 = np.zeros(TOT, dtype=np.int16)
        idx16[lin] = (s % RW).astype(np.int16)          # row within range
        gidx16[c] = idx16.reshape(TOT // 16, 16).T
        pos[c, lin % 128, lin // 128] = (dl % 128).astype(np.float32)
    return dinv, NCH, CHOFF, TOTCH, gidx16, pos


def _build(NCH, CHOFF, TOTCH):
    import concourse.bacc as bacc
    import concourse.bass as bass
    import concourse.mybir as mybir
    import concourse.tile as tile
    from concourse.masks import make_identity

    f32 = mybir.dt.float32
    bf16 = mybir.dt.bfloat16
    i16 = mybir.dt.int16
    RELU = mybir.ActivationFunctionType.Relu
    COPY = mybir.ActivationFunctionType.Copy
    EQ = mybir.AluOpType.is_equal

    # chunks per pair-group and per (pg, r)
    g0_of = [pg * NR * G * 2 for pg in range(NPG)]
    nch_pg = [int(CHOFF[g0_of[pg] + NR * G * 2] - CHOFF[g0_of[pg]])
              for pg in range(NPG)]
    NCHMAX = max(nch_pg)
    NCRMAX = max(int(CHOFF[g0_of[pg] + (r + 1) * G * 2]
                     - CHOFF[g0_of[pg] + r * G * 2])
                 for pg in range(NPG) for r in range(NR))
    TOT16 = TOTCH * CHUNK // 16

    nc = bacc.Bacc("TRN2", target_bir_lowering=False, debug=False,
                   enable_asserts=False, num_devices=NCORES,
                   dynamic_dma_scratch_size=32768, num_swdge_queues=4)

    xT = nc.dram_tensor("xT", [5, SH], bf16, kind="ExternalInput")
    gidx16 = nc.dram_tensor("gidx16", [16, TOT16], i16, kind="ExternalInput")
    posd = nc.dram_tensor("posd", [128, TOTCH], bf16, kind="ExternalInput")
    iota = nc.dram_tensor("iota", [128, 128], bf16, kind="ExternalInput")
    dinv_cols = nc.dram_tensor("dinv_cols", [128, NT], f32, kind="ExternalInput")
    wts = {}
    for nm, shp, dt in [
            ("w1T", [5, 64], bf16), ("w2T", [64, 128], bf16),
            ("w3T", [128, 128], bf16), ("w4T", [128, 128], bf16),
            ("wc1T", [128, 128], bf16), ("wc2T", [128, 128], bf16),
            ("w5T", [128, 60], bf16), ("b1c", [64, 1], f32),
            ("b2c", [128, 1], f32), ("b3c", [128, 1], f32),
            ("b4c", [128, 1], f32), ("b5c", [60, 1], f32),
            ("bc1c", [128, 1], f32), ("bc2c", [128, 1], f32)]:
        wts[nm] = nc.dram_tensor(nm, shp, dt, kind="ExternalInput")
    out = nc.dram_tensor("out", [SH, 60], f32, kind="ExternalOutput")

    with tile.TileContext(nc) as tc:
        with tc.tile_pool(name="w", bufs=1) as wp, \
             tc.tile_pool(name="act", bufs=2) as actp, \
             tc.tile_pool(name="xs", bufs=3) as xsp, \
             tc.tile_pool(name="sm", bufs=4) as smp, \
             tc.tile_pool(name="idx", bufs=3) as idxp, \
             tc.tile_pool(name="oh", bufs=3) as ohp, \
             tc.tile_pool(name="gat", bufs=2) as gatp, \
             tc.tile_pool(name="mm", bufs=2, space="PSUM") as mmp, \
             tc.tile_pool(name="tr", bufs=1, space="PSUM") as trp, \
             tc.tile_pool(name="agg", bufs=4, space="PSUM") as aggp, \
             tc.tile_pool(name="dram", bufs=1, space="DRAM") as dramp:

            W = {}
            for nm in wts:
                W[nm] = wp.tile(list(wts[nm].shape), wts[nm].dtype, tag=nm,
                                name=nm + "_sb")
                nc.sync.dma_start(out=W[nm][:], in_=wts[nm][:])
            dinv_sb = wp.tile([128, NT], f32, tag="dinv", name="dinv_sb")
            nc.sync.dma_start(out=dinv_sb[:], in_=dinv_cols[:])
            iota_sb = wp.tile([128, 128], bf16, tag="iota", name="iota_sb")
            nc.sync.dma_start(out=iota_sb[:], in_=iota[:])
            posb_sb = wp.tile([128, TOTCH], bf16, tag="pos", name="posb_sb")
            nc.sync.dma_start(out=posb_sb[:], in_=posd[:])
            identb = wp.tile([128, 128], bf16, tag="identb", name="identb")
            make_identity(nc, identb[:])
            identf = wp.tile([128, 128], f32, tag="identf", name="identf")
            make_identity(nc, identf[:])

            ag_in = dramp.tile([SH, HID], bf16, name="ag_in")
            ag_out = dramp.tile([N_PAD, HID], bf16, name="ag_out",
                                addr_space="Shared")
            ag_in2 = dramp.tile([SH, HID], bf16, name="ag_in2")
            ag_out2 = dramp.tile([N_PAD, HID], bf16, name="ag_out2",
                                 addr_space="Shared")
            gidx128 = dramp.tile([128, TOT16], i16, name="gidx128")

            # replicate idx stream to 8 partition stripes (8 Q7 cores)
            for k in range(8):
                nc.sync.dma_start(out=gidx128[16 * k:16 * (k + 1), :],
                                  in_=gidx16[:, :])

            slices = [(s, min(512, SH - s)) for s in range(0, SH, 512)]

            def mlp_layer(dst_t, w_t, b_t, src_t, kin, kout, resid=None):
                for s0, sw in slices:
                    ps = mmp.tile([128, 512], f32, space="PSUM", tag="mm")
                    nc.tensor.matmul(ps[:kout, :sw], lhsT=w_t[:],
                                     rhs=src_t[:kin, s0:s0 + sw],
                                     start=True, stop=True)
                    nc.scalar.activation(dst_t[:kout, s0:s0 + sw],
                                         ps[:kout, :sw], RELU, bias=b_t[:])
                    if resid is not None:
                        nc.vector.tensor_add(dst_t[:kout, s0:s0 + sw],
                                             dst_t[:kout, s0:s0 + sw],
                                             resid[:kout, s0:s0 + sw])

            # ---- MLP (feature-major, bf16) ----
            hA = actp.tile([128, SH], bf16, tag="act", name="hA")
            for s0, sw in slices:
                xt = xsp.tile([5, 512], bf16, tag="xs", name="xt")
                nc.sync.dma_start(out=xt[:, :sw], in_=xT[:, s0:s0 + sw])
                ps = mmp.tile([128, 512], f32, space="PSUM", tag="mm")
                nc.tensor.matmul(ps[:64, :sw], lhsT=W["w1T"][:], rhs=xt[:5, :sw],
                                 start=True, stop=True)
                nc.scalar.activation(hA[:64, s0:s0 + sw], ps[:64, :sw], RELU,
                                     bias=W["b1c"][:])
            hB = actp.tile([128, SH], bf16, tag="act", name="hB")
            mlp_layer(hB, W["w2T"], W["b2c"], hA, 64, 128)             # h2
            hC = actp.tile([128, SH], bf16, tag="act", name="hC")
            mlp_layer(hC, W["w3T"], W["b3c"], hB, 128, 128, resid=hB)  # h3
            hD = actp.tile([128, SH], bf16, tag="act", name="hD")
            mlp_layer(hD, W["w4T"], W["b4c"], hC, 128, 128, resid=hC)  # h4

            qrr = 0

            def transform_slice(h_fm, wc_t, agi, s0, sw):
                # transform + dinv[src] scale + transpose, one 512-col slice
                ps = mmp.tile([128, 512], f32, space="PSUM", tag="mm")
                nc.tensor.matmul(ps[:, :sw], lhsT=wc_t[:],
                                 rhs=h_fm[:, s0:s0 + sw], start=True,
                                 stop=True)
                gs = xsp.tile([128, 512], bf16, tag="gs", name="gs")
                nc.scalar.activation(gs[:, :sw], ps[:, :sw], COPY)
                for k in range(sw // 128):
                    t = s0 // 128 + k
                    pt = trp.tile([128, 128], bf16, space="PSUM", tag="trb")
                    nc.tensor.transpose(out=pt[:],
                                        in_=gs[:, k * 128:(k + 1) * 128],
                                        identity=identb[:])
                    gn = smp.tile([128, 128], bf16, tag="gn", name="gn")
                    nc.scalar.activation(gn[:], pt[:], COPY,
                                         scale=dinv_sb[:, t:t + 1])
                    nc.sync.dma_start(out=agi[t * 128:(t + 1) * 128, :],
                                      in_=gn[:])

            def head_slice(hF, s0, sw):
                ps = mmp.tile([128, 512], f32, space="PSUM", tag="mm")
                nc.tensor.matmul(ps[:60, :sw], lhsT=W["w5T"][:],
                                 rhs=hF[:, s0:s0 + sw], start=True, stop=True)
                of = xsp.tile([60, 512], f32, tag="of", name="of")
                nc.vector.tensor_scalar_add(of[:, :sw], ps[:60, :sw],
                                            W["b5c"][:])
                for q in range(0, sw, 128):
                    qw = min(128, sw - q)
                    pt = trp.tile([128, 128], f32, space="PSUM", tag="tr")
                    nc.tensor.transpose(out=pt[:qw, :60], in_=of[:60, q:q + qw],
                                        identity=identf[:60, :60])
                    on = smp.tile([128, 60], f32, tag="on", name="on")
                    nc.vector.tensor_copy(on[:qw, :], pt[:qw, :60])
                    nc.sync.dma_start(out=out[s0 + q:s0 + q + qw, :],
                                      in_=on[:qw, :])

            def agg_phase(bc_c, agi, ago, h_next, post_pg):
                nonlocal qrr
                for pg in range(NPG):
                    g0 = g0_of[pg]
                    c0 = int(CHOFF[g0])
                    nch = nch_pg[pg]
                    npairs = min(G, NP2 - pg * G)
                    tiles = [pg * G * 2 + k for k in range(npairs * 2)]
                    ist = idxp.tile([128, NCHMAX * 8], i16, tag="idx",
                                    name="ist")
                    nc.sync.dma_start(out=ist[:, :nch * 8],
                                      in_=gidx128[:, c0 * 8:(c0 + nch) * 8])
                    gst = gatp.tile([128, NCHMAX, 128], bf16, tag="g",
                                    name="gst")
                    for r in range(NR):
                        cr0 = int(CHOFF[g0 + r * G * 2]) - c0
                        ncr = int(CHOFF[g0 + (r + 1) * G * 2]
                                  - CHOFF[g0 + r * G * 2])
                        nc.gpsimd.dma_gather(
                            gst[:, cr0:cr0 + ncr, :],
                            ago[r * RW:(r + 1) * RW, :],
                            ist[:, cr0 * 8:(cr0 + ncr) * 8],
                            ncr * CHUNK, ncr * CHUNK, HID,
                            queue_num=qrr % 4, single_packet=False)
                        qrr += 1
                    # open PSUM accumulators with the self-loop matmul
                    pa = {}
                    for t in tiles:
                        gl = smp.tile([128, 128], bf16, tag="gl", name="gl")
                        nc.sync.dma_start(out=gl[:],
                                          in_=agi[t * 128:(t + 1) * 128, :])
                        pa[t] = aggp.tile([128, 128], f32, space="PSUM",
                                          tag="agg", name=f"pa{t % 8}")
                        nc.tensor.matmul(pa[t][:], lhsT=identb[:], rhs=gl[:],
                                         start=True, stop=False)
                    # last (r, chunk) per tile to close the accumulation
                    last_of = {}
                    for t in tiles:
                        k = t - pg * G * 2
                        for r in range(NR):
                            g = g0 + r * G * 2 + k
                            if CHOFF[g + 1] > CHOFF[g]:
                                last_of[t] = (r, int(CHOFF[g + 1]) - 1)
                    for r in range(NR):
                        cr0 = int(CHOFF[g0 + r * G * 2]) - c0
                        ncr = int(CHOFF[g0 + (r + 1) * G * 2]
                                  - CHOFF[g0 + r * G * 2])
                        # one-hot for this (pg, r) chunk range
                        oh = ohp.tile([128, NCRMAX * 128], bf16, tag="oh",
                                      name="oh")
                        oh3 = oh[:, :ncr * 128].rearrange("p (c f) -> p c f",
                                                          c=ncr)
                        nc.vector.tensor_tensor(
                            out=oh3,
                            in0=posb_sb[:, c0 + cr0:c0 + cr0 + ncr]
                                .unsqueeze(2).to_broadcast([128, ncr, 128]),
                            in1=iota_sb[:].unsqueeze(1)
                                .to_broadcast([128, ncr, 128]),
                            op=EQ)
                        for t in tiles:
                            k = t - pg * G * 2
                            g = g0 + r * G * 2 + k
                            for ci in range(int(CHOFF[g]), int(CHOFF[g + 1])):
                                lco = ci - c0
                                nc.tensor.matmul(
                                    pa[t][:],
                                    lhsT=oh[:, (lco - cr0) * 128:
                                            (lco - cr0 + 1) * 128],
                                    rhs=gst[:, lco, :].squeeze(),
                                    start=False,
                                    stop=(last_of[t] == (r, ci)))
                    # evacuate: relu(agg * dinv[dst] + bias), feat-major
                    for t in tiles:
                        ev = smp.tile([128, 128], bf16, tag="ev", name="ev")
                        nc.scalar.activation(ev[:], pa[t][:], COPY,
                                             scale=dinv_sb[:, t:t + 1])
                        ptE = trp.tile([128, 128], bf16, space="PSUM",
                                       tag="trb")
                        nc.tensor.transpose(out=ptE[:], in_=ev[:],
                                            identity=identb[:])
                        nc.scalar.activation(h_next[:, t * 128:(t + 1) * 128],
                                             ptE[:], RELU, bias=bc_c[:])
                    if post_pg is not None:
                        post_pg(pg)

            # conv1: transform + single full AllGather + aggregation; conv2's
            # transform rides inside conv1's aggregation loop (slice pg is
            # exactly the 4 tiles pair-group pg just evacuated).
            for s0, sw in slices:
                transform_slice(hD, W["wc1T"], ag_in, s0, sw)
            nc.gpsimd.collective_compute(
                "AllGather", mybir.AluOpType.bypass,
                replica_groups=[list(range(NCORES))],
                ins=[ag_in[:, :].opt()], outs=[ag_out[:, :].opt()])

            hE = actp.tile([128, SH], bf16, tag="act", name="hE")

            def post1(pg):
                s0, sw = slices[pg]
                transform_slice(hE, W["wc2T"], ag_in2, s0, sw)

            agg_phase(W["bc1c"], ag_in, ag_out, hE, post1)

            nc.gpsimd.collective_compute(
                "AllGather", mybir.AluOpType.bypass,
                replica_groups=[list(range(NCORES))],
                ins=[ag_in2[:, :].opt()], outs=[ag_out2[:, :].opt()])

            hF = actp.tile([128, SH], bf16, tag="act", name="hF")

            def post2(pg):
                s0, sw = slices[pg]
                head_slice(hF, s0, sw)

            agg_phase(W["bc2c"], ag_in2, ag_out2, hF, post2)
    nc.compile()
    return nc


def kernel(x, edge_index, W1, b1, W2, b2, W3, b3, W4, b4,
           Wc1, bc1, Wc2, bc2, W5, b5):
    import ml_dtypes
    from concourse.bass_utils import run_bass_kernel_spmd

    bf = ml_dtypes.bfloat16
    x = np.asarray(x, dtype=np.float32)
    key = "k"
    if key not in _cache:
        dinv, NCH, CHOFF, TOTCH, gidx16, posf = _prep(np.asarray(edge_index))
        nc = _build(NCH, CHOFF, TOTCH)
        _cache[key] = (dinv, gidx16, posf, nc)
    dinv, gidx16, posf, nc = _cache[key]

    xp = np.zeros((N_PAD, 5), dtype=np.float32)
    xp[:N_NODES] = x
    iota = np.tile(np.arange(128, dtype=np.float32)[None, :],
                   (128, 1)).astype(bf)
    in_maps = []
    for c in range(NCORES):
        sl = slice(c * SH, (c + 1) * SH)
        m = {
            "xT": np.ascontiguousarray(xp[sl].T).astype(bf),
            "gidx16": gidx16[c],
            "posd": posf[c].astype(__import__("ml_dtypes").bfloat16),
            "iota": iota,
            "dinv_cols": np.ascontiguousarray(
                dinv[sl].reshape(NT, 128).T),
            "w1T": np.ascontiguousarray(np.asarray(W1, np.float32).T).astype(bf),
            "w2T": np.ascontiguousarray(np.asarray(W2, np.float32).T).astype(bf),
            "w3T": np.ascontiguousarray(np.asarray(W3, np.float32).T).astype(bf),
            "w4T": np.ascontiguousarray(np.asarray(W4, np.float32).T).astype(bf),
            "wc1T": np.ascontiguousarray(np.asarray(Wc1, np.float32).T).astype(bf),
            "wc2T": np.ascontiguousarray(np.asarray(Wc2, np.float32).T).astype(bf),
            "w5T": np.ascontiguousarray(np.asarray(W5, np.float32).T).astype(bf),
            "b1c": np.asarray(b1, np.float32)[:, None],
            "b2c": np.asarray(b2, np.float32)[:, None],
            "b3c": np.asarray(b3, np.float32)[:, None],
            "b4c": np.asarray(b4, np.float32)[:, None],
            "b5c": np.asarray(b5, np.float32)[:, None],
            "bc1c": np.asarray(bc1, np.float32)[:, None],
            "bc2c": np.asarray(bc2, np.float32)[:, None],
        }
        in_maps.append(m)
    global last_results
    res = run_bass_kernel_spmd(nc, in_maps, list(range(NCORES)),
                               trace=bool(os.environ.get("KERNEL_TRACE")))
    last_results = res
    outs = [res.results[c]["out"] for c in range(NCORES)]
    return np.concatenate(outs, axis=0)[:N_NODES]
